# revision 29
# baseline (speedup 1.0000x reference)
"""Trainium2 Bass kernel for nn_BL_36721970381090 (dense_mlp).

Math: the reference network collapses (einsum commutation + Kronecker
structure) to, per batch row b:
    z[d, u]   = sum_s fc2_w[u, s] * x[b, d, s]          (200 feats)
    item2[t,u]= sum_d W11[t, d] * z[d, u] + bias1[t, u] (600 feats)
    out[b, o] = sum_{t,u} W12[o, t] * fc4_w[0, u] * relu(item2[t, u]) + bias2[o, 0]

Strategy: pure data parallel over 8 NeuronCores (batch 131072 -> 8 x 16384),
bf16 moving / fp32 psum. Two measured TRN2 constraints drive the design:
(1) matmuls with moving-K <= ~82 run at ~0.9 ns/col vs 0.5 ns/col for
K >= 100, so every matmul keeps K in {100, 120}; (2) every dma_start costs
~0.8 us of serial queue-engine time regardless of size, so DMA count is
minimized and the z->y gathers run on the otherwise idle GpSimd DGE queue.

Batch is processed in groups of two 512-column sub-blocks (1024 cols):
  stage1 (8 mm, K=100, N=512): x chunks; chunk pair p accumulates into
     pz[:,p,:] via block-diagonal stationaries with columns ordered
     (q, dl, u) -> z row q*50 + dl*5 + u.  Two pz tags (even/odd sub-block)
     so consecutive stage1s never wait on the z copies.
  z copy (2 ops [100,1024]) psum -> sbuf bf16, one per sub-block.
  gather (4 gpsimd DMAs [100,512]): pure linearization reshape lands
     y [100, 5, 512]: rows 0:40 = sub-block A d-features, row 40 = ones,
     rows 41:81 = sub-block B d-features, row 81 = ones, 82:100 = pad.
     Each y column holds one A element and one B element stacked (K-pack).
  stage2 (20 mm, K=100, N=256): ps1 [120,2,256] (one PSUM bank per (u,ch))
     = w2[(u,th)] [100,120].T @ y[:,u,ch*256:...]; out rows = (slot,
     t-half 60); the ones rows carry bias1 into the matmul.
  relu (10 ops [120,512]) -> r (u,ch) [120,2,256] bf16.
  stage3 (20 mm, K=120, N=256): ps2[:,ch,:] [6,256] += m2b[(u,th)]
     [120,6].T @ r[:,th,:]  (slot-block-diag: rows 0:3 slot A, 3:6 B;
     both ch halves accumulate in one bank via the 2KB zero-region).
Software pipelining: PE order per group g is [stage1(2g) | stage1(2g+1) |
stage2/relu/stage3(g-1)] so the PE never waits on copies or gathers.
bias2 folded on host after gather.
"""

import numpy as np
import ml_dtypes
from contextlib import ExitStack

import concourse.bass as bass
import concourse.bacc as bacc
import concourse.mybir as mybir
from concourse.bass import ds
from concourse.tile import TileContext
from concourse.bass_utils import run_bass_kernel_spmd

B, D1, D2 = 131072, 40, 10
T0, T1, O0 = 120, 5, 3
NCORES = 8
BC = B // NCORES          # 16384 batch per core
KF = D1 * D2              # 400 input features (d, s)
KC = 100                  # stage-1 K-chunk
NB = 512                  # sub-block batch columns
NH = 256                  # stage2/3 column half
NBD = 2048                # x DMA block (4 x NB)
NSB = BC // NB            # 32 sub-blocks per core
NG = NSB // 2             # 16 groups
TH = T0 // 2              # 60: t half
YP = 100                  # y partitions (82 data/ones + 18 pad)

F32 = mybir.dt.float32
BF16 = mybir.dt.bfloat16
BF = ml_dtypes.bfloat16
RELU = mybir.ActivationFunctionType.Relu
COPY = mybir.ActivationFunctionType.Copy

_CACHE = {}


def _build_nc():
    nc = bacc.Bacc()
    xt = nc.dram_tensor("xt", (KF, BC), BF16, kind="ExternalInput")
    a0_h = nc.dram_tensor("a0", (KC, KC), BF16, kind="ExternalInput")
    a1_h = nc.dram_tensor("a1", (KC, KC), BF16, kind="ExternalInput")
    w2_h = nc.dram_tensor("w2", (YP, T1 * 2 * T0), BF16, kind="ExternalInput")
    m2_h = nc.dram_tensor("m2", (T0, T1 * 2 * 2 * O0), BF16, kind="ExternalInput")
    fill_h = nc.dram_tensor("fill", (20, T1 * NB), BF16, kind="ExternalInput")
    outT = nc.dram_tensor("outT", (O0, BC), F32, kind="ExternalOutput")
    outV = outT.rearrange("o (q s h c) -> o q s h c", s=2, h=2, c=NH)

    with TileContext(nc) as tc, ExitStack() as ctx:
        consts = ctx.enter_context(tc.tile_pool(name="consts", bufs=1))
        a_sb = [consts.tile([KC, KC], BF16, tag=f"a{q}", name=f"a{q}") for q in range(2)]
        nc.sync.dma_start(a_sb[0][:, :], a0_h[:, :])
        nc.sync.dma_start(a_sb[1][:, :], a1_h[:, :])
        w2_sb = consts.tile([YP, T1 * 2 * T0], BF16, tag="w2")
        nc.sync.dma_start(w2_sb[:, :], w2_h[:, :])
        m2_sb = consts.tile([T0, T1 * 2 * 2 * O0], BF16, tag="m2")
        nc.sync.dma_start(m2_sb[:, :], m2_h[:, :])

        xpool = ctx.enter_context(tc.tile_pool(name="xp", bufs=3))
        zpool = ctx.enter_context(tc.tile_pool(name="zp", bufs=2))
        ypool = ctx.enter_context(tc.tile_pool(name="yp", bufs=3))
        rpool = ctx.enter_context(tc.tile_pool(name="rp", bufs=2))
        opool = ctx.enter_context(tc.tile_pool(name="op", bufs=2))
        pzp = ctx.enter_context(tc.tile_pool(name="pz", bufs=1, space="PSUM"))
        ps1p = ctx.enter_context(tc.tile_pool(name="ps1", bufs=3, space="PSUM"))
        ps2p = ctx.enter_context(tc.tile_pool(name="ps2", bufs=1, space="PSUM"))

        # pre-warm ones rows (40, 81) and pad rows (82:100) of each y buffer
        for w in range(3):
            yw = ypool.tile([YP, T1, NB], BF16, tag="y", name=f"ywarm{w}")
            nc.gpsimd.dma_start(yw[ds(D1, 1), :, :], fill_h[ds(0, 1), :])
            nc.gpsimd.dma_start(yw[ds(2 * D1 + 1, 1), :, :], fill_h[ds(0, 1), :])
            nc.gpsimd.dma_start(yw[ds(2 * D1 + 2, YP - 2 * D1 - 2), :, :],
                                fill_h[ds(2, YP - 2 * D1 - 2), :])

        xtiles = {}
        fetched = set()

        def xfetch(blk):
            if blk in fetched or blk >= NSB // (NBD // NB):
                return
            fetched.add(blk)
            tl = [xpool.tile([KC, NBD], BF16, tag=f"x{k}", name=f"xk{k}_{blk}")
                  for k in range(4)]
            for k in range(4):
                nc.gpsimd.dma_start(tl[k][:, :],
                                    xt[ds(k * KC, KC), ds(blk * NBD, NBD)])
            for jj2 in range(NBD // NB):
                xtiles[blk * 4 + jj2] = [t[:, ds(jj2 * NB, NB)] for t in tl]

        def stage1(i):
            """4 matmuls + a z copy for sub-block i; returns the z tile."""
            blk = i // (NBD // NB)
            xfetch(blk)
            if i % (NBD // NB) == 1:
                xfetch(blk + 1)
            xs = xtiles.pop(i)
            pz = pzp.tile([KC, 2, NB], F32, tag=f"pz{i % 2}", name=f"pz{i}")
            for p in range(2):
                nc.tensor.matmul(pz[:, p, :], a_sb[0][:, :], xs[2 * p],
                                 start=True, stop=False)
                nc.tensor.matmul(pz[:, p, :], a_sb[1][:, :], xs[2 * p + 1],
                                 start=False, stop=True)
            z = zpool.tile([KC, 2, NB], BF16, tag=f"z{i % 2}", name=f"z{i}")
            if i % 2 == 0:
                nc.scalar.activation(z[:, :, :], pz[:, :, :], COPY)
            else:
                nc.vector.tensor_copy(z[:, :, :], pz[:, :, :])
            return z

        def gather(g, za, zb):
            y = ypool.tile([YP, T1, NB], BF16, tag="y", name=f"y{g}")
            nc.gpsimd.dma_start(y[ds(0, 20), :, :], za[:, 0, :])
            nc.gpsimd.dma_start(y[ds(20, 20), :, :], za[:, 1, :])
            nc.gpsimd.dma_start(y[ds(D1 + 1, 20), :, :], zb[:, 0, :])
            nc.gpsimd.dma_start(y[ds(D1 + 21, 20), :, :], zb[:, 1, :])
            return y

        def stage23(g, y, osb):
            rts = {}
            for ch in range(2):
                for u in range(T1):
                    pp = ps1p.tile([T0, 2, NH], F32, tag="ps1",
                                   name=f"pp{u}_{ch}_{g}")
                    for th in range(2):
                        nc.tensor.matmul(pp[:, th, :],
                                         w2_sb[:, ds((u * 2 + th) * T0, T0)],
                                         y[:, u, ds(ch * NH, NH)],
                                         start=(th == 0), stop=(th == 1),
                                         skip_group_check=True)
                    r = rpool.tile([T0, 2, NH], BF16, tag=f"r{u}{ch}",
                                   name=f"rt{u}_{ch}_{g}")
                    if (ch * T1 + u) % 2 == 0:
                        nc.vector.tensor_scalar_max(r[:, :, :], pp[:, :, :], 0.0)
                    else:
                        nc.scalar.activation(r[:, :, :], pp[:, :, :], RELU)
                    rts[(u, ch)] = r
            ps2 = ps2p.tile([2 * O0, 2, NH], F32, tag="ps2", name=f"ps2_{g}")
            for ch in range(2):
                n = 0
                for u in range(T1):
                    for th in range(2):
                        nc.tensor.matmul(
                            ps2[:, ch, :],
                            m2_sb[:, ds((u * 2 + th) * 2 * O0, 2 * O0)],
                            rts[(u, ch)][:, th, :],
                            start=(ch == 0 and n == 0), stop=(n == 2 * T1 - 1),
                            skip_group_check=True)
                        n += 1
            nc.vector.tensor_copy(osb[:, g % 2, :, :], ps2[:, :, :])
            if g % 2 == 1:
                blk = g // 2
                nc.sync.dma_start(outV[:, ds(blk * 2, 2), 0, :, :],
                                  osb[ds(0, O0), :, :, :])
                nc.sync.dma_start(outV[:, ds(blk * 2, 2), 1, :, :],
                                  osb[ds(O0, O0), :, :, :])

        osb_map = {}
        pend = None          # (g, ytile)
        for g in range(NG):
            if g % 2 == 0:
                osb_map[g // 2] = opool.tile([2 * O0, 2, 2, NH], F32, tag="osb",
                                             name=f"osb{g // 2}")
            za = stage1(2 * g)
            zb = stage1(2 * g + 1)
            y = gather(g, za, zb)
            if pend is not None:
                stage23(pend[0], pend[1], osb_map[pend[0] // 2])
            pend = (g, y)
        stage23(pend[0], pend[1], osb_map[pend[0] // 2])
    nc.finalize()
    return nc


def _host_prep(W11, fc2_w, bias1, W12, fc4_w):
    # stage-1 stationaries: row rc = dl*10 + s; col m = q*50 + dl*5 + u
    a = np.zeros((2, KC, KC), np.float32)
    for q in range(2):
        for dl in range(10):
            for s in range(10):
                for u in range(T1):
                    a[q, dl * 10 + s, q * 50 + dl * 5 + u] = fc2_w[u, s]
    # stage-2 stationaries [100, (u, th) * 120]: out rows = (slot, t-sub 60)
    w2 = np.zeros((YP, T1 * 2 * T0), np.float32)
    for u in range(T1):
        for th in range(2):
            c0 = (u * 2 + th) * T0
            tsl = slice(th * TH, (th + 1) * TH)
            w2[0:D1, c0 : c0 + TH] = W11.T[:, tsl]
            w2[D1, c0 : c0 + TH] = bias1[tsl, u]
            w2[D1 + 1 : 2 * D1 + 1, c0 + TH : c0 + T0] = W11.T[:, tsl]
            w2[2 * D1 + 1, c0 + TH : c0 + T0] = bias1[tsl, u]
    # stage-3 stationaries [120, (u, th) * 6]: rows (slot, t-sub), cols (slot, o)
    m2 = np.zeros((T0, T1 * 2 * 2 * O0), np.float32)
    for u in range(T1):
        for th in range(2):
            c0 = (u * 2 + th) * 2 * O0
            for sl in range(2):
                for o in range(O0):
                    m2[sl * TH : (sl + 1) * TH, c0 + sl * O0 + o] = (
                        W12[o, th * TH : (th + 1) * TH] * fc4_w[0, u])
    fill = np.ones((20, T1 * NB), np.float32)
    return (a[0].astype(BF), a[1].astype(BF), w2.astype(BF), m2.astype(BF),
            fill.astype(BF))


def kernel(x, W11, fc2_w, bias1, W12, fc4_w, bias2, _trace=False):
    x = np.asarray(x, dtype=np.float32)
    W11 = np.asarray(W11, np.float32)
    fc2_w = np.asarray(fc2_w, np.float32)
    bias1 = np.asarray(bias1, np.float32)
    W12 = np.asarray(W12, np.float32)
    fc4_w = np.asarray(fc4_w, np.float32)
    b2v = np.asarray(bias2, np.float32)[:, 0]

    a0, a1, w2, m2, fill = _host_prep(W11, fc2_w, bias1, W12, fc4_w)

    if "nc" not in _CACHE:
        _CACHE["nc"] = _build_nc()
    nc = _CACHE["nc"]

    in_maps = []
    for c in range(NCORES):
        xs = x[c * BC : (c + 1) * BC]
        xtc = xs.transpose(1, 2, 0).reshape(KF, BC).astype(BF)
        in_maps.append({"xt": xtc, "a0": a0, "a1": a1, "w2": w2, "m2": m2,
                        "fill": fill})

    res = run_bass_kernel_spmd(nc, in_maps, core_ids=list(range(NCORES)), trace=_trace)
    outs = [np.asarray(res.results[c]["outT"], np.float32) for c in range(NCORES)]
    full = np.concatenate(outs, axis=1).T + b2v[None, :]
    if _trace:
        kernel.last_exec_time_ns = res.exec_time_ns
    return full.astype(np.float32)
